# revision 22
# baseline (speedup 1.0000x reference)
"""Causal multi-head attention on 8 Trainium2 NeuronCores.

Sharding: core c handles batch b = c//2 and head-half hg = c%2 (8 of 16
heads, as 4 pairs). Per core: QKV projection (bf16 matmuls, f32 PSUM),
flash-style causal attention in transposed layout (scores_T[t, s], softmax
denominator via a ones-column appended to V), per-pair AllGather of the
normalized attention outputs, and a column-parallel output projection
(w_o columns sharded host-side per core parity). Host reassembles y from
the per-core [m_half, s] transposed outputs.

v2 schedule: attention emits score matmuls in adjacent head-pairs (PE
row-tiling packs the two 64-contraction matmuls) and groups [sc,sc][pv,pv]
per 2 t-tiles to minimize PE tiling-mode switches; PSUM-evacuation copies
run on VectorE (keeping ScalarE exp-only); AllGathers are per (s-tile,
pair) so collectives start early and the final output projection streams
chunk-by-chunk as gathers land.
"""
import sys

sys.path.insert(0, "/opt/trn_rl_repo")

import numpy as np
import ml_dtypes

import concourse.bass as bass
import concourse.mybir as mybir
import concourse.tile as tile
from concourse import bacc
from concourse.bass_utils import run_bass_kernel_spmd

BF16 = ml_dtypes.bfloat16
DT = mybir.dt.bfloat16
F32 = mybir.dt.float32
EXP = mybir.ActivationFunctionType.Exp

B, S, DM, H, DK = 4, 2048, 1024, 16, 64
N_CORES = 8
N_PAIRS = 4          # head pairs per core (8 heads)
N_MCH = DM // 128    # m-chunks of the model dim (contraction for QKV proj)
REPLICA_GROUPS = [[0, 1], [2, 3], [4, 5], [6, 7]]


def build_nc(seq=S, n_pairs=N_PAIRS):
    """Build the SPMD kernel graph. seq must be a multiple of 512."""
    nst = seq // 512          # 512-wide s-tiles
    ntt_all = seq // 128      # 128-wide t-tiles
    nc = bacc.Bacc("TRN2", target_bir_lowering=False, debug=False,
                   num_devices=N_CORES)

    # inputs come pre-tiled host-side: partition-major, contiguous 8KB
    # per-partition lines so every load DMA runs at full line bandwidth
    xT = nc.dram_tensor("xT", [seq // 512, 128, N_MCH, 512], DT,
                        kind="ExternalInput")
    wq = nc.dram_tensor("wq", [128, N_MCH, 128 * n_pairs], DT,
                        kind="ExternalInput")
    wk = nc.dram_tensor("wk", [128, N_MCH, 128 * n_pairs], DT,
                        kind="ExternalInput")
    wv = nc.dram_tensor("wv", [128, N_MCH, 128 * n_pairs], DT,
                        kind="ExternalInput")
    wo = nc.dram_tensor("wo", [128, 2 * n_pairs, 512], DT,
                        kind="ExternalInput")
    mask128 = nc.dram_tensor("mask128", [128, 128], DT, kind="ExternalInput")
    yT = nc.dram_tensor("yT", [512, seq], DT, kind="ExternalOutput")

    n_dch = 2 * n_pairs   # d-chunks of 128 in the gathered attention
    hw = 128 * n_pairs    # head-dim columns per core (2*n_pairs heads x 64)

    with tile.TileContext(nc) as tc:
        with (
            tc.tile_pool(name="dram", bufs=1, space="DRAM") as dram,
            tc.tile_pool(name="persist", bufs=1) as persist,
            tc.tile_pool(name="psum_p", bufs=2, space="PSUM") as pp,
            tc.tile_pool(name="psum_s", bufs=2, space="PSUM") as ps_s,
            tc.tile_pool(name="psum_av", bufs=2, space="PSUM") as ps_av,
            tc.tile_pool(name="pt", bufs=6) as p_pool,
            tc.tile_pool(name="nrm", bufs=2) as nrm,
            tc.tile_pool(name="yc", bufs=2) as ycp,
            tc.tile_pool(name="stg", bufs=3) as stg,
        ):
            ag_in = dram.tile([nst, n_pairs, 2, 64, 512], DT)
            ag_out = dram.tile([nst, n_pairs, 2, 2, 64, 512], DT)
            # last s-tile: pairs 0-2 share one gather so the CC core has
            # no backlog when pair 3's (final, exposed) gather arrives
            ag_in3 = dram.tile([2, 64, 3, 512], DT)
            ag_out3 = dram.tile([2, 2, 64, 3, 512], DT)

            q_sb = persist.tile([128, n_pairs, seq], DT, tag="q")
            k_sb = persist.tile([128, n_pairs, seq], DT, tag="k")
            v_sb = persist.tile([128, ntt_all, 2 * n_pairs, 65], DT, tag="v")
            af_sb = persist.tile([128, n_dch, seq], DT, tag="af")
            m_sb = persist.tile([128, 128], DT, tag="m")
            wo_sb = persist.tile([128, n_dch, 512], DT, tag="wo")
            wq_sb = persist.tile([128, N_MCH, hw], DT, tag="wq")
            wk_sb = persist.tile([128, N_MCH, hw], DT, tag="wk")
            wv_sb = persist.tile([128, N_MCH, hw], DT, tag="wv")
            xt = []
            for st in range(nst):
                t = persist.tile([128, N_MCH, 512], DT, tag=f"xt{st}")
                xt.append(t)

            # First-wave loads (xt0/wv/wq) are chunk-split and spread over
            # all five engine DMA queues so the first projection matmul can
            # start ~11us in; subtile deps let each matmul wait only on its
            # own chunk. Later tiles load as whole-tile DMAs (8KB lines).
            nc.vector.memset(v_sb[:, :, :, 64], 1.0)
            for c in range(4):
                nc.sync.dma_start(out=xt[0][:, c, :], in_=xT[0, :, c, :])
                nc.gpsimd.dma_start(out=xt[0][:, c + 4, :],
                                    in_=xT[0, :, c + 4, :])
            nc.scalar.dma_start(out=m_sb[:], in_=mask128[:])
            for c in range(8):
                nc.scalar.dma_start(out=wv_sb[:, c, :], in_=wv[:, c, :])
            # second wave
            nc.sync.dma_start(out=wq_sb[:, 4:8, :], in_=wq[:, 4:8, :])
            for c in range(4):
                nc.gpsimd.dma_start(out=wq_sb[:, c, :], in_=wq[:, c, :])
            nc.scalar.dma_start(out=wk_sb[:], in_=wk[:])
            # third wave
            nc.sync.dma_start(out=wo_sb[:], in_=wo[:])
            if nst > 1:
                nc.scalar.dma_start(out=xt[1][:], in_=xT[1])
            if nst > 2:
                nc.sync.dma_start(out=xt[2][:], in_=xT[2])
            if nst > 3:
                nc.scalar.dma_start(out=xt[3][:], in_=xT[3])

            yT_v = yT[:].rearrange("(t p) s -> p t s", p=128)

            # ---- emission helpers (each returns a closure doing one
            # PE-dense psum-group; used to fill PE during attention) ----
            # chunk order matched to the first-wave DMA arrival interleave
            C_ORDER = [0, 4, 1, 5, 2, 6, 3, 7]

            def vproj_group(tt):
                def go():
                    st, r = tt // 4, tt % 4
                    ps = pp.tile([128, hw], F32, tag="proj", name=f"psv{tt}")
                    for i, c in enumerate(C_ORDER):
                        nc.tensor.matmul(
                            ps[:],
                            lhsT=xt[st][:, c, r * 128:(r + 1) * 128],
                            rhs=wv_sb[:, c, 0:hw],
                            start=(i == 0), stop=(i == N_MCH - 1))
                    nc.vector.tensor_copy(
                        v_sb[:, tt, :, 0:64],
                        ps[:].rearrange("p (h k) -> p h k", k=64))
                return go

            def qkproj_group(pair, st, which):
                def go():
                    w_sb, dst = ((wq_sb, q_sb), (wk_sb, k_sb))[which]
                    ps = pp.tile([128, 512], F32, tag="proj",
                                 name=f"psqk{pair}_{st}_{which}")
                    for c in range(N_MCH):
                        nc.tensor.matmul(
                            ps[:],
                            lhsT=w_sb[:, c, pair * 128:(pair + 1) * 128],
                            rhs=xt[st][:, c, :],
                            start=(c == 0), stop=(c == N_MCH - 1))
                    nc.vector.tensor_copy(
                        dst[:, pair, st * 512:(st + 1) * 512], ps[:])
                return go

            def outproj_group(mt, st, pool=None):
                def go():
                    ps = (pool or pp).tile([128, 512], F32, tag="proj",
                                           name=f"pso{mt}_{st}")
                    # pair-major chunk order: consume each pair's AG strips
                    # as they land
                    order = [g * n_pairs + p for p in range(n_pairs)
                             for g in range(2)]
                    for i, c in enumerate(order):
                        nc.tensor.matmul(
                            ps[:],
                            lhsT=wo_sb[:, c, mt * 128:(mt + 1) * 128],
                            rhs=af_sb[:, c, st * 512:(st + 1) * 512],
                            start=(i == 0), stop=(i == n_dch - 1))
                    yc = ycp.tile([128, 512], DT, tag="yc", name=f"yc{mt}_{st}")
                    nc.vector.tensor_copy(yc[:], ps[:])
                    # keep sync/gpsimd queues latency-clean for the
                    # normalize chain and AllGather staging
                    nc.scalar.dma_start(
                        out=yT_v[:, mt, st * 512:(st + 1) * 512], in_=yc[:])
                return go

            def proj_groups_for_st(st):
                gs = []
                for tt in range(4 * st, 4 * st + 4):
                    gs.append(vproj_group(tt))
                for pair in range(n_pairs):
                    for which in range(2):
                        gs.append(qkproj_group(pair, st, which))
                return gs

            # ---- attention for one (pair, st) ----
            # slot-grouped: [sc(2i) sc(2i+1)] [pv(2i-2) pv(2i-1)] [filler]
            # so the two 64-contraction score matmuls of each tt pack in
            # the PE (row tiles 0-63 / 64-127) and PE tiling-mode switches
            # happen twice per slot instead of twice per tt.
            def attention(pair, st, filler, stage, pace):
                ntt = 4 * st + 4
                is_last = (st == nst - 1 and pair == n_pairs - 1)
                av0 = ps_av.tile([65, 512], F32, tag="av",
                                 name=f"av0_{pair}_{st}")
                av1 = ps_av.tile([65, 512], F32, tag="av",
                                 name=f"av1_{pair}_{st}")
                av = [av0, av1]
                pts = {}

                def scores(tt):
                    ps = ps_s.tile([128, 2, 512], F32, tag="sc",
                                   name=f"sc{pair}_{st}_{tt}")
                    kk = tt - 4 * st
                    f0 = kk * 128 if kk > 0 else 0
                    for h in range(2):
                        lo = h * 64
                        nc.tensor.matmul(
                            ps[:, h, f0:512],
                            lhsT=k_sb[lo:lo + 64, pair,
                                      tt * 128:(tt + 1) * 128],
                            rhs=q_sb[lo:lo + 64, pair,
                                     st * 512 + f0:(st + 1) * 512],
                            start=True, stop=True)
                    return ps

                def exp_mask(tt, ps):
                    pt = p_pool.tile([128, 2, 512], DT, tag="pt",
                                     name=f"pt{pair}_{st}_{tt}")
                    kk = tt - 4 * st
                    if kk <= 0:
                        nc.scalar.activation(pt[:], ps[:], EXP, scale=0.125)
                    else:
                        nc.scalar.activation(
                            pt[:, :, kk * 128:512],
                            ps[:, :, kk * 128:512], EXP, scale=0.125)
                    if kk >= 0:
                        for h in range(2):
                            nc.vector.tensor_mul(
                                pt[:, h, kk * 128:(kk + 1) * 128],
                                pt[:, h, kk * 128:(kk + 1) * 128],
                                m_sb[:])
                    pts[tt] = pt

                def pv(tt):
                    pt = pts.pop(tt)
                    kk = tt - 4 * st
                    f0 = kk * 128 if kk > 0 else 0
                    for h in range(2):
                        nc.tensor.matmul(
                            av[h][:, f0:512],
                            lhsT=v_sb[:, tt, 2 * pair + h, :],
                            rhs=pt[:, h, f0:512],
                            start=(tt == 0), stop=(tt == ntt - 1))

                def pace_tick():
                    pace["done"] += 1
                    owed = (pace["pops"] * pace["done"]) // pace["total"] \
                        - pace["popped"]
                    while filler and owed > 0:
                        filler.pop(0)()
                        pace["popped"] += 1
                        owed -= 1

                nslots = (ntt + 1) // 2
                for i in range(nslots):
                    pss = []
                    for tt in (2 * i, 2 * i + 1):
                        if tt < ntt:
                            pss.append((tt, scores(tt)))
                    for tt, ps in pss:
                        exp_mask(tt, ps)
                    for tt in (2 * i - 2, 2 * i - 1):
                        if tt >= 0:
                            pv(tt)
                    pace_tick()
                for tt in (ntt - 2, ntt - 1):
                    pv(tt)
                pace_tick()

                # normalize: stage = av[0:64] * (1/denom), multiplying
                # straight out of PSUM. denom row (psum partition 64) ->
                # sbuf -> DMA to partition 0 (the custom-DVE recip and
                # gpsimd broadcast only read partition 0 correctly)
                dtop = nrm.tile([65, 2, 512], F32, tag="dtop",
                                name=f"dtop{pair}_{st}")
                if is_last:
                    # no successor pair needs the av banks: skip the bulk
                    # PSUM->SBUF evacuation, only the denom row moves
                    for h in range(2):
                        nc.vector.tensor_copy(dtop[64:65, h, :],
                                              av[h][64:65, :])
                else:
                    for h in range(2):
                        nc.vector.tensor_copy(dtop[:, h, :], av[h][:])
                den0 = nrm.tile([1, 2, 512], F32, tag="den0",
                                name=f"den0_{pair}_{st}")
                nc.sync.dma_start(out=den0[:], in_=dtop[64:65, :, :])
                r = nrm.tile([1, 2, 512], F32, tag="r", name=f"r{pair}_{st}")
                nc.vector.reciprocal_approx_fast(r[:], den0[:])
                bb = nrm.tile([64, 2, 512], F32, tag="b", name=f"bb{pair}_{st}")
                nc.gpsimd.partition_broadcast(bb[:], r[:])
                for h in range(2):
                    nc.vector.tensor_mul(
                        stage[:, h, :],
                        (av[h][0:64, :] if is_last else dtop[0:64, h, :]),
                        bb[:, h, :])

            # ---------------- main s-tile-outer schedule ----------------
            for tt in range(4):
                vproj_group(tt)()
            for which in range(2):
                qkproj_group(0, 0, which)()
            for st in range(nst):
                # filler budget: next s-tile's projections fill st<nst-1;
                # ALL earlier s-tiles' output projections fill the last
                # s-tile, whose attention is otherwise ScalarE-paced
                filler = []
                if st == 0:
                    for pair in range(1, n_pairs):
                        for which in range(2):
                            filler.append(qkproj_group(pair, 0, which))
                if st + 1 < nst:
                    filler += proj_groups_for_st(st + 1)
                if st == nst - 1:
                    for ost in range(nst - 1):
                        for mt in range(4):
                            filler.append(outproj_group(mt, ost))
                total_iters = n_pairs * ((4 * st + 4 + 1) // 2 + 1)
                pace = {"total": total_iters, "done": 0,
                        "pops": len(filler), "popped": 0}
                merged = st == nst - 1
                for pair in range(n_pairs):
                    stage = stg.tile([64, 2, 512], DT, tag="stage",
                                     name=f"stage{st}_{pair}")
                    attention(pair, st, filler, stage, pace)
                    if merged and pair < 3:
                        for h in range(2):
                            nc.sync.dma_start(
                                out=ag_in3[h, :, pair, :], in_=stage[:, h, :])
                        if pair == 2:
                            nc.gpsimd.collective_compute(
                                "AllGather",
                                mybir.AluOpType.bypass,
                                replica_groups=REPLICA_GROUPS,
                                ins=[ag_in3[:].opt()],
                                outs=[ag_out3[:].opt()],
                            )
                            for g in range(2):
                                for h in range(2):
                                    nc.sync.dma_start(
                                        out=af_sb[h * 64:(h + 1) * 64,
                                                  g * n_pairs:g * n_pairs + 3,
                                                  st * 512:(st + 1) * 512],
                                        in_=ag_out3[g, h])
                        continue
                    # exchange this pair's attention columns immediately
                    for h in range(2):
                        nc.sync.dma_start(
                            out=ag_in[st, pair, h], in_=stage[:, h, :])
                    nc.gpsimd.collective_compute(
                        "AllGather",
                        mybir.AluOpType.bypass,
                        replica_groups=REPLICA_GROUPS,
                        ins=[ag_in[st, pair].opt()],
                        outs=[ag_out[st, pair].opt()],
                    )
                    for g in range(2):
                        for h in range(2):
                            nc.sync.dma_start(
                                out=af_sb[h * 64:(h + 1) * 64,
                                          g * n_pairs + pair,
                                          st * 512:(st + 1) * 512],
                                in_=ag_out[st, pair, g, h])
                while filler:
                    filler.pop(0)()
            # final s-tile's output projection: stream all 4 mt-groups
            # pair-by-pair as the AG strips land (mt2/mt3 psum comes from
            # the scores pool, free once the last exp is done)
            last = nst - 1
            fps = []
            for mt in range(4):
                pool = pp if mt < 2 else ps_s
                fps.append(pool.tile([128, 512], F32, tag="proj" if mt < 2 else "sc",
                                     name=f"psfin{mt}"))
            for p in range(n_pairs):
                for mt in range(4):
                    for g in range(2):
                        c = g * n_pairs + p
                        nc.tensor.matmul(
                            fps[mt][:],
                            lhsT=wo_sb[:, c, mt * 128:(mt + 1) * 128],
                            rhs=af_sb[:, c, last * 512:(last + 1) * 512],
                            start=(p == 0 and g == 0),
                            stop=(p == n_pairs - 1 and g == 1))
            for mt in range(4):
                yc = ycp.tile([128, 512], DT, tag="yc", name=f"ycfin{mt}")
                nc.vector.tensor_copy(yc[:], fps[mt][:])
                nc.scalar.dma_start(
                    out=yT_v[:, mt, last * 512:(last + 1) * 512], in_=yc[:])
    nc.compile()
    return nc


def _make_mask128():
    p = np.arange(128)[:, None]
    f = np.arange(128)[None, :]
    return (p <= f).astype(BF16)


_NC_CACHE = {}


def _get_nc(seq=S, n_pairs=N_PAIRS):
    key = (seq, n_pairs)
    if key not in _NC_CACHE:
        _NC_CACHE[key] = build_nc(seq, n_pairs)
    return _NC_CACHE[key]


def _tile_pcn(w):
    # [DM, n] -> [128, N_MCH, n] with [p, c, n] = w[c*128+p, n]
    return np.ascontiguousarray(
        w.reshape(N_MCH, 128, w.shape[1]).transpose(1, 0, 2)).astype(BF16)


def make_in_maps(x, w_qkv, w_o):
    masks = _make_mask128()
    in_maps = []
    for c in range(N_CORES):
        b, hg = c // 2, c % 2
        heads = slice(hg * 8, hg * 8 + 8)
        # x[b].T is [DM, S]; pre-tile to [nst, 128, N_MCH, 512] with
        # [st, p, c, s] = xT[c*128+p, st*512+s]
        xb = x[b].T.reshape(N_MCH, 128, S // 512, 512)
        xt = np.ascontiguousarray(xb.transpose(2, 1, 0, 3)).astype(BF16)
        in_maps.append({
            "xT": xt,
            "wq": _tile_pcn(w_qkv[0, heads].reshape(512, DM).T),
            "wk": _tile_pcn(w_qkv[1, heads].reshape(512, DM).T),
            "wv": _tile_pcn(w_qkv[2, heads].reshape(512, DM).T),
            "wo": _tile_pcn(w_o[hg * 512:(hg + 1) * 512, :].T),
            "mask128": masks,
        })
    return in_maps


def kernel(x, w_qkv, w_o):
    x = np.asarray(x, dtype=np.float32)
    w_qkv = np.asarray(w_qkv, dtype=np.float32)
    w_o = np.asarray(w_o, dtype=np.float32)

    nc = _get_nc()
    in_maps = make_in_maps(x, w_qkv, w_o)
    res = run_bass_kernel_spmd(nc, in_maps, list(range(N_CORES)), trace=False)

    y = np.empty((B, S, DM), dtype=np.float32)
    for c in range(N_CORES):
        b, hg = c // 2, c % 2
        y[b, :, hg * 512:(hg + 1) * 512] = \
            res.results[c]["yT"].T.astype(np.float32)
    return y
